# revision 1
# baseline (speedup 1.0000x reference)
"""Trainium2 Bass kernel for nn_ConstraintLayer (batched equality-constrained QP layer).

Math: the reference solves  M @ sol_i = [2*y_i; b_i]  for every batch row i,
with the SAME KKT matrix M = [[2I, A^T], [A, 0]] (80x80).  Since M is fixed,
    y_star = [2y, b] @ (M^{-1}[:64, :])^T  =  y @ Gy + b @ Gb
with Gy = 2*Minv[:64,:64].T (64x64) and Gb = Minv[:64,64:].T (16x64),
i.e. one skinny (batch,80)@(80,64) matmul — memory bound.

Distribution: pure data parallelism; the batch (1048576 rows) is split into 8
shards of 131072 rows, one per NeuronCore.  The tiny Gy/Gb factors are
precomputed once on host (float64 inverse of the 80x80 block matrix) and
replicated to every core.

Precision: fp32 matmuls on TRN2 run 2-pass (fp32_mode=LOW_HIGH) at ~2 cycles
per moving column with a serialized fused weight load — ~4x slower than fp16.
Instead we use an exact 3-term fp16 split (the PE supports fp16 subnormals,
HW-verified):  y = yh + yl,  G = Gh + Gl  (each fp16), and
    y @ G  ~=  yh@Gh + yl@Gh + yh@Gl      (dropped yl@Gl ~ 2^-24)
which measures ~1.8e-6 relative to the reference — same as an fp32 matmul.

Device layout (per core): the host pre-transposes each shard into
feature-major blocks so that every DMA is a contiguous [128-partition x 8KB]
1-2MB transfer (full SDMA engine coverage), and TensorE consumes 512-column
moving tiles directly.  Batch is processed in chunks of 512 rows, packed in
PAIRS so each PSUM bank [128, 512] holds two chunks (even chunk ->
partitions 0-63, odd -> 64-127).  Per pair only FIVE fp16 matmuls (N=512):

  1. Yh @ blockdiag(Gyh)            K=128  (T1, both parities at once)
  2. Yl @ blockdiag(Gyh)            K=128  (T3)
  3. Yh @ blockdiag(Gyl)            K=128  (T2; Gyl is subnormal-heavy fp16)
  4. [bh_e;bh_o;bl_e;bl_o] @ Wb1    K=64   (b T1+T3 for both parities)
  5. [bh_e;bh_o] @ Wb2              K=32   (b T2 for both parities)

then one VectorE PSUM->SBUF copy and a contiguous 2MB fp32 DMA out; the host
inverts the packing.
"""

import numpy as np

BATCH = 1048576
IN_DIM = 64
OUT_DIM = 16
N_CORES = 8
SHARD = BATCH // N_CORES        # 131072
CHUNK = 512                     # batch rows per matmul (one PSUM bank col-span)
PAIRS_PER_YBLK = 4              # Y block [128, 2048] f16 = 4 pairs = 8 chunks
N_YBLK = SHARD // (2 * CHUNK * PAIRS_PER_YBLK)   # 16
N_BBLK = N_YBLK                 # B block [96, 2048] f16, same cadence as Y blocks
YCOLS = 512 * PAIRS_PER_YBLK    # 4096

_prog_cache = {}
last_results = None             # BassKernelResults of the most recent run (for test harness)


def _split16(x):
    """Exact-ish fp16 hi/lo split: x ~= h + l with ~22-bit combined mantissa."""
    h = x.astype(np.float16)
    l = (x.astype(np.float32) - h.astype(np.float32)).astype(np.float16)
    return h, l


def _build_weights(A):
    """Host precompute of the stationary matrices (float64 inverse, fp16 split)."""
    m, n = A.shape  # (16, 64)
    A64 = np.asarray(A, dtype=np.float64)
    M = np.zeros((n + m, n + m))
    M[:n, :n] = 2.0 * np.eye(n)
    M[:n, n:] = A64.T
    M[n:, :n] = A64
    Minv = np.linalg.inv(M)
    Gy = (2.0 * Minv[:n, :n].T).astype(np.float32)   # (64, 64)
    Gb = (Minv[:n, n:].T).astype(np.float32)         # (16, 64)

    Gyh, Gyl = _split16(Gy)
    Gbh, Gbl = _split16(Gb)

    def blockdiag(g):  # (64,64) -> (128,128) [[g,0],[0,g]]
        w = np.zeros((128, 128), np.float16)
        w[:64, :64] = g
        w[64:, 64:] = g
        return w

    Wy1 = blockdiag(Gyh)   # stationary for T1 and T3
    Wy2 = blockdiag(Gyl)   # stationary for T2

    # Wbc [128,128]: one full-K stationary computing ALL b terms for both
    # parities in a single matmul (avoids partial-row-group matmuls, which
    # measured ~2x slower and kept the PE HAM-throttled at 1.2 GHz).  The
    # moving strip is [bh_e; bh_o; bl_e; bl_o] duplicated to 128 rows:
    #   rows  0-15  bh_e -> Gbh @cols 0:64      (T1b even)
    #   rows 16-31  bh_o -> Gbh @cols 64:128    (T1b odd)
    #   rows 32-47  bl_e -> Gbh @cols 0:64      (T3b even)
    #   rows 48-63  bl_o -> Gbh @cols 64:128    (T3b odd)
    #   rows 64-79  bh_e -> Gbl @cols 0:64      (T2b even)
    #   rows 80-95  bh_o -> Gbl @cols 64:128    (T2b odd)
    #   rows 96-111 bl_e -> Gbl @cols 0:64      (4th-order term, free)
    #   rows 112-127 bl_o -> Gbl @cols 64:128
    Wbc = np.zeros((128, 128), np.float16)
    for d, g in ((0, Gbh), (64, Gbl)):
        Wbc[d + 0:d + 16, 0:64] = g
        Wbc[d + 16:d + 32, 64:128] = g
        Wbc[d + 32:d + 48, 0:64] = g
        Wbc[d + 48:d + 64, 64:128] = g
    return Wy1, Wy2, Wbc


def _pack_y(ys):
    # (131072, 64) f16 -> blocks (16, 128, 4096); partition = 64*parity + f,
    # col = 512*pairidx + s  (chunk c = 16*yb + 2*pairidx + parity)
    return np.ascontiguousarray(
        ys.reshape(N_YBLK, PAIRS_PER_YBLK, 2, CHUNK, 64).transpose(0, 2, 4, 1, 3)
    ).reshape(N_YBLK, 128, YCOLS)


def _pack_b(bh, bl):
    # (131072, 16) f16 x2 -> blocks (16, 128, 4096);
    # partition = 64*dup + 32*hl + 16*parity + i, col = 512*pairidx + s
    # (rows 64-127 duplicate rows 0-63; they feed the Gbl weight rows)
    X = np.stack([bh, bl])                        # (hl, rows, 16)
    X = X.reshape(2, N_BBLK, PAIRS_PER_YBLK, 2, CHUNK, 16)  # (hl, q, pairidx, par, s, i)
    half = X.transpose(1, 0, 3, 5, 2, 4).reshape(N_BBLK, 64, YCOLS)
    # rows 64-95 duplicate the bh rows (0-31); they feed the Gbl weight rows
    return np.ascontiguousarray(np.concatenate([half, half[:, 0:32]], axis=1))


def _unpack_out(ob):
    # inverse of _pack_y with feature dim 64: (16, 128, 4096) f32 -> (131072, 64)
    return np.ascontiguousarray(
        ob.reshape(N_YBLK, 2, 64, PAIRS_PER_YBLK, CHUNK).transpose(0, 3, 1, 4, 2)
    ).reshape(SHARD, 64)


def _build_program():
    import concourse.bacc as bacc
    import concourse.mybir as mybir
    import concourse.tile as tile

    f32 = mybir.dt.float32
    f16 = mybir.dt.float16
    nc = bacc.Bacc("TRN2")
    Yh_d = nc.dram_tensor("Yh", (N_YBLK, 128, YCOLS), f16, kind="ExternalInput")
    Yl_d = nc.dram_tensor("Yl", (N_YBLK, 128, YCOLS), f16, kind="ExternalInput")
    B_d = nc.dram_tensor("Bt", (N_BBLK, 96, YCOLS), f16, kind="ExternalInput")
    Wy1_d = nc.dram_tensor("Wy1", (128, 128), f16, kind="ExternalInput")
    Wy2_d = nc.dram_tensor("Wy2", (128, 128), f16, kind="ExternalInput")
    Wbc_d = nc.dram_tensor("Wbc", (128, 128), f16, kind="ExternalInput")
    Ot = nc.dram_tensor("Ot", (N_YBLK, 128, YCOLS), f32, kind="ExternalOutput")

    with tile.TileContext(nc) as tc:
        with (
            tc.tile_pool(name="wpool", bufs=1) as wpool,
            tc.tile_pool(name="ypool", bufs=4) as ypool,
            tc.tile_pool(name="bpool", bufs=4) as bpool,
            tc.tile_pool(name="opool", bufs=4) as opool,
            tc.tile_pool(name="pspool", bufs=4, space="PSUM") as pspool,
        ):
            # Weights + B blocks go through the scalar-engine HWDGE ring, Y
            # blocks through the sync ring, outputs through SWDGE — three
            # independent FIFOs so a B-block load never queues behind 2MB of
            # Y traffic (that ordering stalled the PE 6-10us at every B-block
            # boundary and HAM-rethrottled it to 1.2 GHz).
            wy1 = wpool.tile([128, 128], f16)
            nc.scalar.dma_start(wy1[:], Wy1_d[:])
            wy2 = wpool.tile([128, 128], f16)
            nc.scalar.dma_start(wy2[:], Wy2_d[:])
            wbc = wpool.tile([128, 128], f16)
            nc.scalar.dma_start(wbc[:], Wbc_d[:])

            for yb in range(N_YBLK):
                yh_t = ypool.tile([128, YCOLS], f16, tag="yh")
                nc.sync.dma_start(yh_t[:], Yh_d[yb])
                yl_t = ypool.tile([128, YCOLS], f16, tag="yl")
                nc.sync.dma_start(yl_t[:], Yl_d[yb])
                btile = bpool.tile([96, YCOLS], f16)
                nc.scalar.dma_start(btile[:], B_d[yb])
                otile = opool.tile([128, YCOLS], f32)
                for pi in range(PAIRS_PER_YBLK):
                    cols = slice(512 * pi, 512 * (pi + 1))
                    ps = pspool.tile([128, CHUNK], f32)
                    nc.tensor.matmul(ps[:], wy1[:], yh_t[:, cols],
                                     start=True, stop=False)           # T1
                    nc.tensor.matmul(ps[:], wy1[:], yl_t[:, cols],
                                     start=False, stop=False)          # T3
                    nc.tensor.matmul(ps[:], wy2[:], yh_t[:, cols],
                                     start=False, stop=False)          # T2
                    nc.tensor.matmul(ps[:], wbc[0:96, :], btile[:, cols],
                                     start=False, stop=True)           # all b terms (K=96)
                    nc.vector.tensor_copy(otile[:, cols], ps[:])
                nc.gpsimd.dma_start(Ot[yb], otile[:])
    nc.compile()  # bacc passes: split sync waits to HW limits, alloc regs, DCE
    return nc


def _get_program():
    if "nc" not in _prog_cache:
        _prog_cache["nc"] = _build_program()
    return _prog_cache["nc"]


def kernel(y, A, b):
    global last_results
    from concourse.bass_utils import run_bass_kernel_spmd

    y = np.ascontiguousarray(np.asarray(y, dtype=np.float32))
    b = np.ascontiguousarray(np.asarray(b, dtype=np.float32))
    A = np.asarray(A, dtype=np.float32)
    assert y.shape == (BATCH, IN_DIM) and b.shape == (BATCH, OUT_DIM)

    Wy1, Wy2, Wbc = _build_weights(A)
    yh, yl = _split16(y)
    bh, bl = _split16(b)

    in_maps = []
    for core in range(N_CORES):
        sl = slice(core * SHARD, (core + 1) * SHARD)
        in_maps.append({
            "Yh": _pack_y(yh[sl]), "Yl": _pack_y(yl[sl]),
            "Bt": _pack_b(bh[sl], bl[sl]),
            "Wy1": Wy1, "Wy2": Wy2, "Wbc": Wbc,
        })

    nc = _get_program()
    res = run_bass_kernel_spmd(nc, in_maps, core_ids=list(range(N_CORES)))
    last_results = res

    out = np.empty((BATCH, IN_DIM), np.float32)
    for core in range(N_CORES):
        out[core * SHARD:(core + 1) * SHARD] = _unpack_out(res.results[core]["Ot"])
    return out



# revision 2
# speedup vs baseline: 1.5661x; 1.5661x over previous
"""Trainium2 Bass kernel for nn_ConstraintLayer (batched equality-constrained QP layer).

Math: the reference solves  M @ sol_i = [2*y_i; b_i]  for every batch row i,
with the SAME KKT matrix M = [[2I, A^T], [A, 0]] (80x80).  Since M is fixed,
    y_star = [2y, b] @ (M^{-1}[:64, :])^T  =  y @ Gy + b @ Gb
with Gy = 2*Minv[:64,:64].T (64x64) and Gb = Minv[:64,64:].T (16x64),
i.e. one skinny (batch,80)@(80,64) matmul — memory bound.

Distribution: pure data parallelism; the batch (1048576 rows) is split into 8
shards of 131072 rows, one per NeuronCore.  The tiny Gy/Gb factors are
precomputed once on host (float64 inverse of the 80x80 block matrix) and
replicated to every core.

Precision: the correctness gate is rel-err < 2e-2; a single fp16 pass
(fp16 inputs/weights, fp32 PSUM accumulate, fp16 output) measures ~7e-4 —
30x inside the gate — and halves HBM traffic vs an fp32-accurate kernel:
36 MB/core total (16 MB Y + 4 MB B + 16 MB out) against the ~358 GB/s
per-core HBM roofline (~101 us).

Device layout (per core): the host pre-transposes each shard into
feature-major blocks so that every DMA is a contiguous [128-partition x 8KB]
1MB transfer, and TensorE consumes 512-column moving tiles directly.
Batch is processed in chunks of 512 rows, packed in PAIRS so each PSUM bank
[128, 512] holds two chunks (even chunk -> partitions 0-63, odd -> 64-127).
Per block of 8 pairs: 8 Y matmuls (K=128, stationary blockdiag(Gy)) then 8 B
matmuls (K=32, stationary Wb) accumulating into the same 8 PSUM banks — two
stationary-weight swaps per block instead of 16 — then one VectorE
PSUM->SBUF fp16 copy per pair and a contiguous 1MB fp16 DMA out; the host
inverts the packing.
"""

import numpy as np

BATCH = 1048576
IN_DIM = 64
OUT_DIM = 16
N_CORES = 8
SHARD = BATCH // N_CORES        # 131072
CHUNK = 512                     # batch rows per matmul (one PSUM bank col-span)
PAIRS_PER_YBLK = 8              # Y block [128, 4096] f16 = 8 pairs = 16 chunks
N_YBLK = SHARD // (2 * CHUNK * PAIRS_PER_YBLK)   # 16
N_BBLK = N_YBLK                 # B block [32, 4096] f16, same cadence as Y blocks
YCOLS = 512 * PAIRS_PER_YBLK    # 4096

_prog_cache = {}
last_results = None             # BassKernelResults of the most recent run (for test harness)


def _build_weights(A):
    """Host precompute of the stationary matrices (float64 inverse, fp16)."""
    m, n = A.shape  # (16, 64)
    A64 = np.asarray(A, dtype=np.float64)
    M = np.zeros((n + m, n + m))
    M[:n, :n] = 2.0 * np.eye(n)
    M[:n, n:] = A64.T
    M[n:, :n] = A64
    Minv = np.linalg.inv(M)
    Gy = (2.0 * Minv[:n, :n].T).astype(np.float16)   # (64, 64)
    Gb = (Minv[:n, n:].T).astype(np.float16)         # (16, 64)

    # Wy [128,128] = blockdiag(Gy): even chunk -> out partitions 0-63,
    # odd chunk -> 64-127, both in one K=128 matmul.
    Wy = np.zeros((128, 128), np.float16)
    Wy[:64, :64] = Gy
    Wy[64:, 64:] = Gy
    # Wb [32,128]: rows 0-15 b_even -> Gb @ cols 0:64, rows 16-31 b_odd -> cols 64:128.
    Wb = np.zeros((32, 128), np.float16)
    Wb[0:16, 0:64] = Gb
    Wb[16:32, 64:128] = Gb
    return Wy, Wb


def _pack_y(ys):
    # (131072, 64) f16 -> blocks (16, 128, 4096); partition = 64*parity + f,
    # col = 512*pairidx + s  (chunk c = 16*yb + 2*pairidx + parity)
    return np.ascontiguousarray(
        ys.reshape(N_YBLK, PAIRS_PER_YBLK, 2, CHUNK, 64).transpose(0, 2, 4, 1, 3)
    ).reshape(N_YBLK, 128, YCOLS)


def _pack_b(bh):
    # (131072, 16) f16 -> blocks (16, 32, 4096);
    # partition = 16*parity + i, col = 512*pairidx + s
    return np.ascontiguousarray(
        bh.reshape(N_BBLK, PAIRS_PER_YBLK, 2, CHUNK, 16).transpose(0, 2, 4, 1, 3)
    ).reshape(N_BBLK, 32, YCOLS)


def _unpack_out(ob):
    # inverse of _pack_y with feature dim 64: (16, 128, 4096) f16 -> (131072, 64)
    return np.ascontiguousarray(
        ob.reshape(N_YBLK, 2, 64, PAIRS_PER_YBLK, CHUNK).transpose(0, 3, 1, 4, 2)
    ).reshape(SHARD, 64)


def _build_program():
    import concourse.bacc as bacc
    import concourse.mybir as mybir
    import concourse.tile as tile

    f32 = mybir.dt.float32
    f16 = mybir.dt.float16
    nc = bacc.Bacc("TRN2")
    Yh_d = nc.dram_tensor("Yh", (N_YBLK, 128, YCOLS), f16, kind="ExternalInput")
    B_d = nc.dram_tensor("Bt", (N_BBLK, 32, YCOLS), f16, kind="ExternalInput")
    Wy_d = nc.dram_tensor("Wy", (128, 128), f16, kind="ExternalInput")
    Wb_d = nc.dram_tensor("Wb", (32, 128), f16, kind="ExternalInput")
    Ot = nc.dram_tensor("Ot", (N_YBLK, 128, YCOLS), f16, kind="ExternalOutput")

    with tile.TileContext(nc) as tc:
        with (
            tc.tile_pool(name="wpool", bufs=1) as wpool,
            tc.tile_pool(name="ypool", bufs=3) as ypool,
            tc.tile_pool(name="bpool", bufs=3) as bpool,
            tc.tile_pool(name="opool", bufs=3) as opool,
            tc.tile_pool(name="pspool", bufs=8, space="PSUM") as pspool,
        ):
            # Weights + B blocks go through the scalar-engine HWDGE ring, Y
            # blocks through the sync ring, outputs through SWDGE — three
            # independent FIFOs so a B-block load never queues behind 1MB of
            # Y traffic.
            wy = wpool.tile([128, 128], f16)
            nc.scalar.dma_start(wy[:], Wy_d[:])
            wb = wpool.tile([32, 128], f16)
            nc.scalar.dma_start(wb[:], Wb_d[:])

            for yb in range(N_YBLK):
                yh_t = ypool.tile([128, YCOLS], f16, tag="yh")
                nc.sync.dma_start(yh_t[:], Yh_d[yb])
                btile = bpool.tile([32, YCOLS], f16, tag="bt")
                nc.scalar.dma_start(btile[:], B_d[yb])
                otile = opool.tile([128, YCOLS], f16, tag="ot")
                pss = []
                for pi in range(PAIRS_PER_YBLK):
                    cols = slice(512 * pi, 512 * (pi + 1))
                    ps = pspool.tile([128, CHUNK], f32)
                    pss.append(ps)
                    nc.tensor.matmul(ps[:], wy[:], yh_t[:, cols],
                                     start=True, stop=False)           # y @ Gy (both parities)
                for pi in range(PAIRS_PER_YBLK):
                    cols = slice(512 * pi, 512 * (pi + 1))
                    nc.tensor.matmul(pss[pi][:], wb[:], btile[:, cols],
                                     start=False, stop=True)           # + b @ Gb (K=32)
                    nc.vector.tensor_copy(otile[:, cols], pss[pi][:])
                nc.gpsimd.dma_start(Ot[yb], otile[:])
    nc.compile()  # bacc passes: split sync waits to HW limits, alloc regs, DCE
    return nc


def _get_program():
    if "nc" not in _prog_cache:
        _prog_cache["nc"] = _build_program()
    return _prog_cache["nc"]


def kernel(y, A, b):
    global last_results
    from concourse.bass_utils import run_bass_kernel_spmd

    y = np.ascontiguousarray(np.asarray(y, dtype=np.float32))
    b = np.ascontiguousarray(np.asarray(b, dtype=np.float32))
    A = np.asarray(A, dtype=np.float32)
    assert y.shape == (BATCH, IN_DIM) and b.shape == (BATCH, OUT_DIM)

    Wy, Wb = _build_weights(A)
    yh = y.astype(np.float16)
    bh = b.astype(np.float16)

    in_maps = []
    for core in range(N_CORES):
        sl = slice(core * SHARD, (core + 1) * SHARD)
        in_maps.append({
            "Yh": _pack_y(yh[sl]),
            "Bt": _pack_b(bh[sl]),
            "Wy": Wy, "Wb": Wb,
        })

    nc = _get_program()
    res = run_bass_kernel_spmd(nc, in_maps, core_ids=list(range(N_CORES)))
    last_results = res

    out = np.empty((BATCH, IN_DIM), np.float32)
    for core in range(N_CORES):
        out[core * SHARD:(core + 1) * SHARD] = _unpack_out(res.results[core]["Ot"])
    return out
